# revision 18
# baseline (speedup 1.0000x reference)
"""Trainium2 Bass kernel for nn_InceptionTraversal (hierarchical sphere-softmax
MoE routing + per-band sigmoid routers).

Strategy (v2 — bf16 single-pass-per-phase redesign)
---------------------------------------------------
Math: routing_k  ∝  exp(-T3_k) * R_jl(k) * (4 + sth_k), normalized over k.
  T3_k = alpha*(d1+d2+d3) + lam*(sqrt(d1)+sqrt(d2)+sqrt(d3))  (path sums)
  R_jl = 1/(Z2_j * Z3_jl),  Z = per-parent softmax denominators
  sth_k = sum_n tanh(r_nk/2)     (sigmoid routers, 0.5 folds cancel)
Z1, the 1/8 refr scale and all constant folds cancel in the final normalize.

Device plan (per core, 16384 tokens, 128-token groups, 4-group superchunks):
  * All matmuls bf16 (4x faster than fp32 LOW/HIGH on the PE, FWL weight
    loads).  Distance precision is restored with a hi/lo split: psi rows are
    shipped as [psi_hi; psi_hi; psi_lo] and the distance weight rows as
    [Wd_hi; Wd_lo; Wd_hi], so x = psi@Wd is exact to ~bf16^2 (validated
    5e-3 end-to-end vs 2e-2 tolerance).
  * Phase A (sqrt ACT table): matmul K=30 N=84 -> x for all 84 spheres;
    ACT sqrt -> u = lam*sqrt(d+eps) (bf16); DVE path-sums U3 = u1+u2+u3
    per leaf -> bf16 stash (DVE is otherwise idle in this phase).
  * Phase B (exp/tanh table): matmul K=94 N=400:
      cols [0:80)    x_d for spheres 4..84 (E = exp(-x) -> Z2/Z3)
      cols [80:144)  xS = alpha*(d1+d2+d3) per leaf (path-sum weights)
      cols [144:400) r router logits, band-major (n,k)
    ACT: E, th = tanh(r), H = exp(-(xS+U3)); DVE: T3 add, Z-reduces,
    reciprocal, band tree, (4+sth)*m3e with fused row-sum, normalize.
  * All intermediates bf16 (except the reciprocal path, fp32), output bf16
    upcast on host.
Sharding: pure data-parallel over 8 cores (tokens split 8 ways).
"""

import sys

import numpy as np

if "/opt/trn_rl_repo" not in sys.path:
    sys.path.insert(0, "/opt/trn_rl_repo")

# ---- problem constants (hardcoded per contest contract) ----
N_DOM, N_SUB, N_CON = 4, 4, 4
SPECTRAL_DIM, N_BANDS = 64, 4
BAND_SIZE = SPECTRAL_DIM // N_BANDS
TEMP, LAM, EPS = 1.0, 0.1, 1e-8
ALPHA = 1.0 / (2.0 * TEMP * TEMP + EPS)
N_CORES = 8
B, S = 16, 8192
NTOK = B * S
TPC = NTOK // N_CORES          # tokens per core = 16384
GRP = 128                      # tokens per matmul group
G = 4                          # groups per superchunk (PSUM ping-pong)
NSC = TPC // (GRP * G)         # superchunks = 32
NS = 84                        # spheres (4 + 16 + 64)
NLEAF = 64
NR = 256                       # router logits, band-major (n,k)
KD = 10                        # psi features [x2,xy,xz,y2,yz,z2,x,y,z,1]
KA = 3 * KD                    # phase-A rows: [psi_hi; psi_hi; psi_lo]
KB = KA + SPECTRAL_DIM         # phase-B rows: + spectral (bf16)
NE = 80                        # E cols (spheres 4..84)
NB = NE + NLEAF + NR           # phase-B matmul N = 400

_compiled = {}


def _bf16(x):
    x = np.asarray(x, np.float32)
    i = x.view(np.uint32)
    r = ((i >> 16) + ((i >> 15) & 1)).astype(np.uint32) << 16
    return r.view(np.float32)


def _host_matrices(centers1, centers2, centers3, portal1_T, portal2_T,
                   W_bands, b_bands, band_weights):
    """Build WA [30,84] (phase A), WB [94,400] (phase B), both bf16-valued
    fp32 arrays, plus the band-weight info."""
    c1 = centers1.astype(np.float64)
    c2 = centers2.astype(np.float64)
    c3 = centers3.astype(np.float64)
    A1 = portal1_T[:, :, :3].astype(np.float64)
    b1 = portal1_T[:, :, 3].astype(np.float64)
    A2 = portal2_T[:, :, :3].astype(np.float64)
    b2 = portal2_T[:, :, 3].astype(np.float64)

    Ms = np.zeros((NS, 3, 3))
    us = np.zeros((NS, 3))
    s = 0
    for j in range(N_DOM):                     # level 1
        Ms[s] = np.eye(3)
        us[s] = -c1[j]
        s += 1
    for j in range(N_DOM):                     # level 2
        for l in range(N_SUB):
            Ms[s] = A1[j]
            us[s] = b1[j] - c2[j * N_SUB + l]
            s += 1
    for j in range(N_DOM):                     # level 3
        for l in range(N_SUB):
            jl = j * N_SUB + l
            M = A2[jl] @ A1[j]
            v = A2[jl] @ b1[j] + b2[jl]
            for m in range(N_CON):
                Ms[s] = M
                us[s] = v - c3[jl * N_CON + m]
                s += 1
    assert s == NS

    # x_s(p) = psi(p) . Wd[:, s],  psi = [x2,xy,xz,y2,yz,z2,x,y,z,1]
    Wd = np.zeros((KD, NS))
    for i in range(NS):
        Q = Ms[i].T @ Ms[i]
        lin = 2.0 * (Ms[i].T @ us[i])
        Wd[:, i] = [Q[0, 0], 2 * Q[0, 1], 2 * Q[0, 2], Q[1, 1], 2 * Q[1, 2],
                    Q[2, 2], lin[0], lin[1], lin[2], us[i] @ us[i]]
    Wd *= ALPHA                                # PSUM x = alpha * d_true

    # per-leaf path sums: WdS[:, k] = Wd1[j] + Wd2[jl] + Wd3[jlm]
    WdS = np.zeros((KD, NLEAF))
    for j in range(N_DOM):
        for l in range(N_SUB):
            jl = j * N_SUB + l
            for m in range(N_CON):
                k = jl * N_CON + m
                WdS[:, k] = Wd[:, j] + Wd[:, 4 + jl] + Wd[:, 20 + k]

    def hl3(W):  # hi/lo 3-block for [psi_hi; psi_hi; psi_lo] rows
        hi = _bf16(W)
        lo = _bf16(W - hi)
        return np.concatenate([hi, lo, hi], axis=0)

    WA = np.zeros((KA, NS), np.float32)
    WA[:] = hl3(Wd)

    WB = np.zeros((KB, NB), np.float32)
    WB[0:KA, 0:NE] = hl3(Wd[:, 4:NS])
    WB[0:KA, NE:NE + NLEAF] = hl3(WdS)
    # router cols, band-major: col NE+64 + n*64 + k = 0.5*(x_n.W[n,:,k] + b[n,k])
    Wr = np.zeros((SPECTRAL_DIM, NR))
    for n in range(N_BANDS):
        Wr[n * BAND_SIZE:(n + 1) * BAND_SIZE, n * NLEAF:(n + 1) * NLEAF] = \
            0.5 * W_bands[n].astype(np.float64)
    WB[KA:KB, NE + NLEAF:NB] = _bf16(Wr)
    # bias via the psi const row (psi_hi row 9 == 1.0)
    WB[KD - 1, NE + NLEAF:NB] = _bf16(
        0.5 * b_bands.astype(np.float64).reshape(NR))

    w = np.exp(band_weights.astype(np.float64))
    w = w / w.sum()
    equal_w = bool(np.allclose(w, w[0], rtol=1e-6, atol=1e-9))
    return WA, WB, equal_w, w.astype(np.float32)


BLK = 4                        # superchunks per block (DVE batch unit)
NBLK = NSC // BLK              # 8 blocks per core
BCH = BLK * G * GRP            # 2048 tokens per block


def _host_phi(pos_3d, spectral_color):
    """phi [94, NTOK] f32 (bf16-valued): [psi_hi(10); psi_hi(10); psi_lo(10);
    spectral(64)]."""
    p = pos_3d.reshape(-1, 3).astype(np.float32)
    x, y, z = p[:, 0], p[:, 1], p[:, 2]
    psi = np.empty((KD, NTOK), dtype=np.float32)
    psi[0] = x * x
    psi[1] = x * y
    psi[2] = x * z
    psi[3] = y * y
    psi[4] = y * z
    psi[5] = z * z
    psi[6] = x
    psi[7] = y
    psi[8] = z
    psi[9] = 1.0
    hi = _bf16(psi)
    lo = _bf16(psi - hi)
    phi = np.empty((KB, NTOK), dtype=np.float32)
    phi[0:KD] = hi
    phi[KD:2 * KD] = hi
    phi[2 * KD:KA] = lo
    phi[KA:KB] = _bf16(spectral_color.reshape(-1, SPECTRAL_DIM).T)
    return np.ascontiguousarray(phi)


def _build_module(equal_w, w_vec):
    import concourse.bacc as bacc
    import concourse.mybir as mybir
    import concourse.tile as tile

    f32 = mybir.dt.float32
    bf = mybir.dt.bfloat16
    AF = mybir.ActivationFunctionType
    OP = mybir.AluOpType

    nc = bacc.Bacc("TRN2", target_bir_lowering=False)
    phi_d = nc.dram_tensor("phi", [NBLK, KB, BCH], bf, kind="ExternalInput")
    wa_d = nc.dram_tensor("wa", [KA, NS], bf, kind="ExternalInput")
    wb_d = nc.dram_tensor("wb", [KB, NB], bf, kind="ExternalInput")
    out_d = nc.dram_tensor("routing", [TPC, NLEAF], bf, kind="ExternalOutput")

    sq_scale = (LAM * LAM) / ALPHA          # u = sqrt(sq_scale*x + sq_bias)
    sq_bias = LAM * LAM * EPS
    CH = G * GRP                            # 512 tokens per superchunk
    SB = BLK * G                            # 16 group-slots per block

    # activation() turns float biases into const APs — register ours.
    for cval in (sq_bias,):
        if (f32, cval) not in nc.const_aps.aps:
            ct = nc.alloc_sbuf_tensor(f"const-f32-{cval}", [128, 1], f32)
            nc.gpsimd.memset(ct.ap(), cval)
            nc.const_aps.aps[(f32, cval)] = ct.ap()
    nc.all_engine_barrier()

    with tile.TileContext(nc) as tc:
        with (
            tc.tile_pool(name="const", bufs=1) as constp,
            tc.tile_pool(name="stash", bufs=1) as stashp,
            tc.tile_pool(name="io", bufs=3) as iop,
            tc.tile_pool(name="work", bufs=3) as wp,
            tc.tile_pool(name="ps", bufs=2, space="PSUM") as psp,
        ):
            wa_sb = constp.tile([KA, NS], bf)
            nc.sync.dma_start(wa_sb[:], wa_d[:])
            wb_sb = constp.tile([KB, NB], bf)
            nc.sync.dma_start(wb_sb[:], wb_d[:])

            u3_stash = stashp.tile([GRP, NSC * G * NLEAF], bf)

            # ---------------- Phase A: sqrt table set ----------------
            # phi is small enough to keep fully resident: load each block
            # once here; phase B does no DMA at all.
            phi_res = [stashp.tile([KB, BCH], bf, name=f"phir{b}",
                                   tag=f"phir{b}")
                       for b in range(NBLK)]
            # psi rows first (phase A waits only on these) ...
            for blk in range(NBLK):
                nc.sync.dma_start(phi_res[blk][0:KA, :], phi_d[blk, 0:KA, :])
            for blk in range(NBLK):
                phiA = phi_res[blk]
                # ... spectral rows stream in behind phase A compute
                nc.sync.dma_start(phiA[KA:KB, :], phi_d[blk, KA:KB, :])
                u4 = wp.tile([GRP, SB, NS], bf, tag="u4")
                for h in range(BLK):
                    psA = psp.tile([GRP, G, 512], f32, tag="ps")
                    for g in range(G):
                        nc.tensor.matmul(
                            psA[:, g, 0:NS],
                            phiA[0:KA, (h * G + g) * GRP:(h * G + g + 1) * GRP],
                            wa_sb[:],
                            start=True, stop=True,
                        )
                    nc.scalar.activation(
                        u4[:, h * G:(h + 1) * G, :], psA[:, :, 0:NS],
                        AF.Sqrt, bias=sq_bias, scale=sq_scale)
                U2 = wp.tile([GRP, SB, 16], bf, tag="U2")
                nc.vector.tensor_tensor(
                    U2.rearrange("p s (j l) -> p s j l", l=4),
                    u4[:, :, 4:20].rearrange("p s (j l) -> p s j l", l=4),
                    u4[:, :, 0:4].unsqueeze(3).broadcast_to((GRP, SB, 4, 4)),
                    OP.add)
                ust = u3_stash[:, blk * (SB * NLEAF):(blk + 1) * (SB * NLEAF)]
                nc.vector.tensor_tensor(
                    ust.rearrange("p (s jl m) -> p s jl m", s=SB, m=4),
                    u4[:, :, 20:NS].rearrange("p s (jl m) -> p s jl m", m=4),
                    U2[:].unsqueeze(3).broadcast_to((GRP, SB, 16, 4)),
                    OP.add)

            # No inter-phase barrier: each engine's queue is FIFO, and all
            # sqrt ACTIVATEs are emitted before any exp/tanh ones, so the
            # act-table loads stay correct while the phases pipeline.

            # ---------------- Phase B: exp/tanh table set ----------------
            for blk in range(NBLK):
                phiB = phi_res[blk]
                # eU3 = exp(-U3): depends only on the phase-A stash, so emit
                # it first — the DVE H-chain then waits only on the last EH.
                ust = u3_stash[:, blk * (SB * NLEAF):(blk + 1) * (SB * NLEAF)]
                eU3 = wp.tile([GRP, SB, NLEAF], bf, tag="eU3")
                nc.scalar.activation(
                    eU3[:], ust.rearrange("p (s k) -> p s k", s=SB),
                    AF.Exp, scale=-1.0)
                EH4 = wp.tile([GRP, SB, NE + NLEAF], bf, tag="EH4")
                th4 = wp.tile([GRP, N_BANDS, SB, NLEAF], bf, tag="th4")
                for h in range(BLK):
                    psB = psp.tile([GRP, G, 512], f32, tag="ps")
                    for g in range(G):
                        nc.tensor.matmul(
                            psB[:, g, 0:NB],
                            phiB[:, (h * G + g) * GRP:(h * G + g + 1) * GRP],
                            wb_sb[:],
                            start=True, stop=True,
                        )

                    # th: band-major cols -> [p, band, slot, k] layout
                    nc.scalar.activation(
                        th4[:, :, h * G:(h + 1) * G, :],
                        psB[:, :, NE + NLEAF:NB].rearrange(
                            "p g (n k) -> p n g k", n=N_BANDS),
                        AF.Tanh)

                    # E = exp(-x_d), Hx = exp(-xS): one ACTIVATE, PSUM-only dep
                    nc.scalar.activation(
                        EH4[:, h * G:(h + 1) * G, :],
                        psB[:, :, 0:NE + NLEAF], AF.Exp, scale=-1.0)

                E = EH4[:, :, 0:NE]
                Hx = EH4[:, :, NE:NE + NLEAF]
                H = wp.tile([GRP, SB, NLEAF], bf, tag="H")
                nc.vector.tensor_tensor(H[:], Hx, eU3[:], OP.mult)

                # fused Z2|Z3 reduce: E cols [0:16)=level2, [16:80)=level3
                Z = wp.tile([GRP, SB, 20], f32, tag="Z")
                nc.vector.tensor_reduce(
                    Z[:], E.rearrange("p s (q m) -> p s q m", m=4),
                    mybir.AxisListType.X, OP.add)
                D = wp.tile([GRP, SB, 16], f32, tag="D")
                nc.vector.tensor_tensor(
                    D.rearrange("p s (j l) -> p s j l", l=4),
                    Z[:, :, 4:20].rearrange("p s (j l) -> p s j l", l=4),
                    Z[:, :, 0:4].unsqueeze(3).broadcast_to((GRP, SB, 4, 4)),
                    OP.mult)
                R = wp.tile([GRP, SB, 16], f32, tag="R")
                nc.vector.reciprocal_approx_fast(R[:], D[:])

                m3e = wp.tile([GRP, SB, NLEAF], bf, tag="m3e")
                nc.vector.tensor_tensor(
                    m3e.rearrange("p s (jl m) -> p s jl m", m=4),
                    H[:].rearrange("p s (jl m) -> p s jl m", m=4),
                    R[:].unsqueeze(3).broadcast_to((GRP, SB, 16, 4)),
                    OP.mult)

                # band tree over contiguous [p, SB*64] planes
                if equal_w:
                    thw = th4
                else:
                    thw = wp.tile([GRP, N_BANDS, SB, NLEAF], bf, tag="thw")
                    for n in range(N_BANDS):
                        nc.vector.tensor_scalar_mul(
                            thw[:, n], th4[:, n], float(w_vec[n] * N_BANDS))
                z01 = wp.tile([GRP, SB, NLEAF], bf, tag="z01")
                nc.vector.tensor_tensor(z01[:], thw[:, 0], thw[:, 1], OP.add)
                z23 = wp.tile([GRP, SB, NLEAF], bf, tag="z23")
                nc.vector.tensor_tensor(z23[:], thw[:, 2], thw[:, 3], OP.add)
                sth = wp.tile([GRP, SB, NLEAF], bf, tag="sth")
                nc.vector.tensor_tensor(sth[:], z01[:], z23[:], OP.add)

                pre = wp.tile([GRP, SB, NLEAF], bf, tag="pre")
                nc.vector.scalar_tensor_tensor(
                    pre[:], sth[:], 4.0, m3e[:], OP.add, OP.mult)
                ssum = wp.tile([GRP, SB], f32, tag="ssum")
                nc.vector.tensor_reduce(
                    ssum[:], pre[:], mybir.AxisListType.X, OP.add)
                rcp = wp.tile([GRP, SB], f32, tag="rcp")
                nc.vector.reciprocal_approx_fast(rcp[:], ssum[:])

                ot = wp.tile([GRP, SB, NLEAF], bf, tag="ot")
                nc.vector.tensor_tensor(
                    ot[:], pre[:],
                    rcp[:].unsqueeze(2).broadcast_to((GRP, SB, NLEAF)),
                    OP.mult)

                nc.sync.dma_start(
                    out_d[blk * BCH:(blk + 1) * BCH, :].rearrange(
                        "(s p) k -> p s k", p=GRP),
                    ot[:])

    nc.finalize()
    return nc


def _get_compiled(equal_w, w_vec):
    key = (equal_w, tuple(np.round(w_vec.astype(np.float64), 9)))
    if key not in _compiled:
        _compiled[key] = _build_module(equal_w, w_vec)
    return _compiled[key]


def _make_in_maps(pos_3d, spectral_color, centers1, centers2, centers3,
                  portal1_T, portal2_T, W_bands, b_bands, band_weights):
    import ml_dtypes
    WA, WB, equal_w, w_vec = _host_matrices(
        np.asarray(centers1), np.asarray(centers2), np.asarray(centers3),
        np.asarray(portal1_T), np.asarray(portal2_T),
        np.asarray(W_bands), np.asarray(b_bands), np.asarray(band_weights))
    phi = _host_phi(np.asarray(pos_3d), np.asarray(spectral_color))
    bfd = ml_dtypes.bfloat16
    WAb = WA.astype(bfd)
    WBb = WB.astype(bfd)
    phib = phi.astype(bfd)
    in_maps = []
    for c in range(N_CORES):
        pc = phib[:, c * TPC:(c + 1) * TPC]           # [KB, TPC]
        # block-major: [NBLK, KB, BCH] — each block contiguous in DRAM
        pblk = np.ascontiguousarray(
            pc.reshape(KB, NBLK, BCH).transpose(1, 0, 2))
        in_maps.append({
            "phi": pblk,
            "wa": WAb,
            "wb": WBb,
        })
    return in_maps, equal_w, w_vec


def kernel(pos_3d, spectral_color, centers1, centers2, centers3,
           portal1_T, portal2_T, W_bands, b_bands, band_weights):
    from concourse.bass_utils import run_bass_kernel_spmd

    in_maps, equal_w, w_vec = _make_in_maps(
        pos_3d, spectral_color, centers1, centers2, centers3,
        portal1_T, portal2_T, W_bands, b_bands, band_weights)
    nc = _get_compiled(equal_w, w_vec)
    res = run_bass_kernel_spmd(nc, in_maps, core_ids=list(range(N_CORES)))
    outs = [np.asarray(res.results[c]["routing"], dtype=np.float32)
            for c in range(N_CORES)]
    full = np.concatenate(outs, axis=0).reshape(B, S, SPECTRAL_DIM)
    return full.astype(np.float32)


if __name__ == "__main__":
    sys.path.insert(0, "/root/problem")
    import reference
    inputs = {k: np.asarray(v) for k, v in reference.setup_inputs().items()}
    out = kernel(**inputs)
    exp = np.asarray(reference.reference(**inputs))
    err = np.max(np.abs(out - exp)) / max(np.max(np.abs(exp)), 1e-12)
    print("Relative error:", err)


# revision 19
# speedup vs baseline: 1.0128x; 1.0128x over previous
"""Trainium2 Bass kernel for nn_InceptionTraversal (hierarchical sphere-softmax
MoE routing + per-band sigmoid routers).

Strategy (v2 — bf16 single-pass-per-phase redesign)
---------------------------------------------------
Math: routing_k  ∝  exp(-T3_k) * R_jl(k) * (4 + sth_k), normalized over k.
  T3_k = alpha*(d1+d2+d3) + lam*(sqrt(d1)+sqrt(d2)+sqrt(d3))  (path sums)
  R_jl = 1/(Z2_j * Z3_jl),  Z = per-parent softmax denominators
  sth_k = sum_n tanh(r_nk/2)     (sigmoid routers, 0.5 folds cancel)
Z1, the 1/8 refr scale and all constant folds cancel in the final normalize.

Device plan (per core, 16384 tokens, 128-token groups, 4-group superchunks):
  * All matmuls bf16 (4x faster than fp32 LOW/HIGH on the PE, FWL weight
    loads).  Distance precision is restored with a hi/lo split: psi rows are
    shipped as [psi_hi; psi_hi; psi_lo] and the distance weight rows as
    [Wd_hi; Wd_lo; Wd_hi], so x = psi@Wd is exact to ~bf16^2 (validated
    5e-3 end-to-end vs 2e-2 tolerance).
  * Phase A (sqrt ACT table): matmul K=30 N=84 -> x for all 84 spheres;
    ACT sqrt -> u = lam*sqrt(d+eps) (bf16); DVE path-sums U3 = u1+u2+u3
    per leaf -> bf16 stash (DVE is otherwise idle in this phase).
  * Phase B (exp/tanh table): matmul K=94 N=400:
      cols [0:80)    x_d for spheres 4..84 (E = exp(-x) -> Z2/Z3)
      cols [80:144)  xS = alpha*(d1+d2+d3) per leaf (path-sum weights)
      cols [144:400) r router logits, band-major (n,k)
    ACT: E, th = tanh(r), H = exp(-(xS+U3)); DVE: T3 add, Z-reduces,
    reciprocal, band tree, (4+sth)*m3e with fused row-sum, normalize.
  * All intermediates bf16 (except the reciprocal path, fp32), output bf16
    upcast on host.
Sharding: pure data-parallel over 8 cores (tokens split 8 ways).
"""

import sys

import numpy as np

if "/opt/trn_rl_repo" not in sys.path:
    sys.path.insert(0, "/opt/trn_rl_repo")

# ---- problem constants (hardcoded per contest contract) ----
N_DOM, N_SUB, N_CON = 4, 4, 4
SPECTRAL_DIM, N_BANDS = 64, 4
BAND_SIZE = SPECTRAL_DIM // N_BANDS
TEMP, LAM, EPS = 1.0, 0.1, 1e-8
ALPHA = 1.0 / (2.0 * TEMP * TEMP + EPS)
N_CORES = 8
B, S = 16, 8192
NTOK = B * S
TPC = NTOK // N_CORES          # tokens per core = 16384
GRP = 128                      # tokens per matmul group
G = 4                          # groups per superchunk (PSUM ping-pong)
NSC = TPC // (GRP * G)         # superchunks = 32
NS = 84                        # spheres (4 + 16 + 64)
NLEAF = 64
NR = 256                       # router logits, band-major (n,k)
KD = 10                        # psi features [x2,xy,xz,y2,yz,z2,x,y,z,1]
KA = 3 * KD                    # phase-A rows: [psi_hi; psi_hi; psi_lo]
KB = KA + SPECTRAL_DIM         # phase-B rows: + spectral (bf16)
NE = 80                        # E cols (spheres 4..84)
NB = NE + NLEAF + NR           # phase-B matmul N = 400

_compiled = {}


def _bf16(x):
    x = np.asarray(x, np.float32)
    i = x.view(np.uint32)
    r = ((i >> 16) + ((i >> 15) & 1)).astype(np.uint32) << 16
    return r.view(np.float32)


def _host_matrices(centers1, centers2, centers3, portal1_T, portal2_T,
                   W_bands, b_bands, band_weights):
    """Build WA [30,84] (phase A), WB [94,400] (phase B), both bf16-valued
    fp32 arrays, plus the band-weight info."""
    c1 = centers1.astype(np.float64)
    c2 = centers2.astype(np.float64)
    c3 = centers3.astype(np.float64)
    A1 = portal1_T[:, :, :3].astype(np.float64)
    b1 = portal1_T[:, :, 3].astype(np.float64)
    A2 = portal2_T[:, :, :3].astype(np.float64)
    b2 = portal2_T[:, :, 3].astype(np.float64)

    Ms = np.zeros((NS, 3, 3))
    us = np.zeros((NS, 3))
    s = 0
    for j in range(N_DOM):                     # level 1
        Ms[s] = np.eye(3)
        us[s] = -c1[j]
        s += 1
    for j in range(N_DOM):                     # level 2
        for l in range(N_SUB):
            Ms[s] = A1[j]
            us[s] = b1[j] - c2[j * N_SUB + l]
            s += 1
    for j in range(N_DOM):                     # level 3
        for l in range(N_SUB):
            jl = j * N_SUB + l
            M = A2[jl] @ A1[j]
            v = A2[jl] @ b1[j] + b2[jl]
            for m in range(N_CON):
                Ms[s] = M
                us[s] = v - c3[jl * N_CON + m]
                s += 1
    assert s == NS

    # x_s(p) = psi(p) . Wd[:, s],  psi = [x2,xy,xz,y2,yz,z2,x,y,z,1]
    Wd = np.zeros((KD, NS))
    for i in range(NS):
        Q = Ms[i].T @ Ms[i]
        lin = 2.0 * (Ms[i].T @ us[i])
        Wd[:, i] = [Q[0, 0], 2 * Q[0, 1], 2 * Q[0, 2], Q[1, 1], 2 * Q[1, 2],
                    Q[2, 2], lin[0], lin[1], lin[2], us[i] @ us[i]]
    Wd *= ALPHA                                # PSUM x = alpha * d_true

    # per-leaf path sums: WdS[:, k] = Wd1[j] + Wd2[jl] + Wd3[jlm]
    WdS = np.zeros((KD, NLEAF))
    for j in range(N_DOM):
        for l in range(N_SUB):
            jl = j * N_SUB + l
            for m in range(N_CON):
                k = jl * N_CON + m
                WdS[:, k] = Wd[:, j] + Wd[:, 4 + jl] + Wd[:, 20 + k]

    def hl3(W):  # hi/lo 3-block for [psi_hi; psi_hi; psi_lo] rows
        hi = _bf16(W)
        lo = _bf16(W - hi)
        return np.concatenate([hi, lo, hi], axis=0)

    WA = np.zeros((KA, NS), np.float32)
    WA[:] = hl3(Wd)

    WB = np.zeros((KB, NB), np.float32)
    WB[0:KA, 0:NE] = hl3(Wd[:, 4:NS])
    WB[0:KA, NE:NE + NLEAF] = hl3(WdS)
    # router cols, band-major: col NE+64 + n*64 + k = 0.5*(x_n.W[n,:,k] + b[n,k])
    Wr = np.zeros((SPECTRAL_DIM, NR))
    for n in range(N_BANDS):
        Wr[n * BAND_SIZE:(n + 1) * BAND_SIZE, n * NLEAF:(n + 1) * NLEAF] = \
            0.5 * W_bands[n].astype(np.float64)
    WB[KA:KB, NE + NLEAF:NB] = _bf16(Wr)
    # bias via the psi const row (psi_hi row 9 == 1.0)
    WB[KD - 1, NE + NLEAF:NB] = _bf16(
        0.5 * b_bands.astype(np.float64).reshape(NR))

    w = np.exp(band_weights.astype(np.float64))
    w = w / w.sum()
    equal_w = bool(np.allclose(w, w[0], rtol=1e-6, atol=1e-9))
    return WA, WB, equal_w, w.astype(np.float32)


BLK = 4                        # superchunks per block (DVE batch unit)
NBLK = NSC // BLK              # 8 blocks per core
BCH = BLK * G * GRP            # 2048 tokens per block


def _host_phi(pos_3d, spectral_color):
    """phi [94, NTOK] f32 (bf16-valued): [psi_hi(10); psi_hi(10); psi_lo(10);
    spectral(64)]."""
    p = pos_3d.reshape(-1, 3).astype(np.float32)
    x, y, z = p[:, 0], p[:, 1], p[:, 2]
    psi = np.empty((KD, NTOK), dtype=np.float32)
    psi[0] = x * x
    psi[1] = x * y
    psi[2] = x * z
    psi[3] = y * y
    psi[4] = y * z
    psi[5] = z * z
    psi[6] = x
    psi[7] = y
    psi[8] = z
    psi[9] = 1.0
    hi = _bf16(psi)
    lo = _bf16(psi - hi)
    phi = np.empty((KB, NTOK), dtype=np.float32)
    phi[0:KD] = hi
    phi[KD:2 * KD] = hi
    phi[2 * KD:KA] = lo
    phi[KA:KB] = _bf16(spectral_color.reshape(-1, SPECTRAL_DIM).T)
    return np.ascontiguousarray(phi)


def _build_module(equal_w, w_vec):
    import concourse.bacc as bacc
    import concourse.mybir as mybir
    import concourse.tile as tile

    f32 = mybir.dt.float32
    bf = mybir.dt.bfloat16
    AF = mybir.ActivationFunctionType
    OP = mybir.AluOpType

    nc = bacc.Bacc("TRN2", target_bir_lowering=False)
    phi_d = nc.dram_tensor("phi", [NBLK, KB, BCH], bf, kind="ExternalInput")
    wa_d = nc.dram_tensor("wa", [KA, NS], bf, kind="ExternalInput")
    wb_d = nc.dram_tensor("wb", [KB, NB], bf, kind="ExternalInput")
    out_d = nc.dram_tensor("routing", [TPC, NLEAF], bf, kind="ExternalOutput")

    sq_scale = (LAM * LAM) / ALPHA          # u = sqrt(sq_scale*x + sq_bias)
    sq_bias = LAM * LAM * EPS
    CH = G * GRP                            # 512 tokens per superchunk
    SB = BLK * G                            # 16 group-slots per block

    # activation() turns float biases into const APs — register ours.
    for cval in (sq_bias,):
        if (f32, cval) not in nc.const_aps.aps:
            ct = nc.alloc_sbuf_tensor(f"const-f32-{cval}", [128, 1], f32)
            nc.gpsimd.memset(ct.ap(), cval)
            nc.const_aps.aps[(f32, cval)] = ct.ap()
    nc.all_engine_barrier()

    with tile.TileContext(nc) as tc:
        with (
            tc.tile_pool(name="const", bufs=1) as constp,
            tc.tile_pool(name="stash", bufs=1) as stashp,
            tc.tile_pool(name="io", bufs=3) as iop,
            tc.tile_pool(name="work", bufs=3) as wp,
            tc.tile_pool(name="ps", bufs=2, space="PSUM") as psp,
        ):
            wa_sb = constp.tile([KA, NS], bf)
            nc.sync.dma_start(wa_sb[:], wa_d[:])
            wb_sb = constp.tile([KB, NB], bf)
            nc.sync.dma_start(wb_sb[:], wb_d[:])

            u3_stash = stashp.tile([GRP, NSC * G * NLEAF], bf)

            # ---------------- Phase A: sqrt table set ----------------
            # phi is small enough to keep fully resident: load each block
            # once here; phase B does no DMA at all.
            phi_res = [stashp.tile([KB, BCH], bf, name=f"phir{b}",
                                   tag=f"phir{b}")
                       for b in range(NBLK)]
            # psi rows first (phase A waits only on these) ...
            for blk in range(NBLK):
                nc.sync.dma_start(phi_res[blk][0:KA, :], phi_d[blk, 0:KA, :])
            for blk in range(NBLK):
                phiA = phi_res[blk]
                # ... spectral rows stream in behind phase A compute
                nc.sync.dma_start(phiA[KA:KB, :], phi_d[blk, KA:KB, :])
                u4 = wp.tile([GRP, SB, NS], bf, tag="u4")
                for h in range(BLK):
                    psA = psp.tile([GRP, G, 512], f32, tag="ps")
                    for g in range(G):
                        nc.tensor.matmul(
                            psA[:, g, 0:NS],
                            phiA[0:KA, (h * G + g) * GRP:(h * G + g + 1) * GRP],
                            wa_sb[:],
                            start=True, stop=True,
                        )
                    nc.scalar.activation(
                        u4[:, h * G:(h + 1) * G, :], psA[:, :, 0:NS],
                        AF.Sqrt, bias=sq_bias, scale=sq_scale)
                U2 = wp.tile([GRP, SB, 16], bf, tag="U2")
                nc.vector.tensor_tensor(
                    U2.rearrange("p s (j l) -> p s j l", l=4),
                    u4[:, :, 4:20].rearrange("p s (j l) -> p s j l", l=4),
                    u4[:, :, 0:4].unsqueeze(3).broadcast_to((GRP, SB, 4, 4)),
                    OP.add)
                ust = u3_stash[:, blk * (SB * NLEAF):(blk + 1) * (SB * NLEAF)]
                nc.vector.tensor_tensor(
                    ust.rearrange("p (s jl m) -> p s jl m", s=SB, m=4),
                    u4[:, :, 20:NS].rearrange("p s (jl m) -> p s jl m", m=4),
                    U2[:].unsqueeze(3).broadcast_to((GRP, SB, 16, 4)),
                    OP.add)

            # No inter-phase barrier: each engine's queue is FIFO, and all
            # sqrt ACTIVATEs are emitted before any exp/tanh ones, so the
            # act-table loads stay correct while the phases pipeline.

            # ---------------- Phase B: exp/tanh table set ----------------
            for blk in range(NBLK):
                phiB = phi_res[blk]
                # eU3 = exp(-U3): depends only on the phase-A stash, so emit
                # it first — the DVE H-chain then waits only on the last EH.
                ust = u3_stash[:, blk * (SB * NLEAF):(blk + 1) * (SB * NLEAF)]
                eU3 = wp.tile([GRP, SB, NLEAF], bf, tag="eU3")
                nc.scalar.activation(
                    eU3[:], ust.rearrange("p (s k) -> p s k", s=SB),
                    AF.Exp, scale=-1.0)
                EH4 = wp.tile([GRP, SB, NE + NLEAF], bf, tag="EH4")
                th4 = wp.tile([GRP, N_BANDS, SB, NLEAF], bf, tag="th4")
                for h in range(BLK):
                    psB = psp.tile([GRP, G, 512], f32, tag="ps")
                    for g in range(G):
                        nc.tensor.matmul(
                            psB[:, g, 0:NB],
                            phiB[:, (h * G + g) * GRP:(h * G + g + 1) * GRP],
                            wb_sb[:],
                            start=True, stop=True,
                        )

                    # th: band-major cols -> [p, band, slot, k] layout
                    nc.scalar.activation(
                        th4[:, :, h * G:(h + 1) * G, :],
                        psB[:, :, NE + NLEAF:NB].rearrange(
                            "p g (n k) -> p n g k", n=N_BANDS),
                        AF.Tanh)

                    # E = exp(-x_d), Hx = exp(-xS): one ACTIVATE, PSUM-only dep
                    nc.scalar.activation(
                        EH4[:, h * G:(h + 1) * G, :],
                        psB[:, :, 0:NE + NLEAF], AF.Exp, scale=-1.0)

                def epilogue(s0, s1):
                    ns = s1 - s0
                    E = EH4[:, s0:s1, 0:NE]
                    Hx = EH4[:, s0:s1, NE:NE + NLEAF]
                    eU = eU3[:, s0:s1, :]
                    H = wp.tile([GRP, ns, NLEAF], bf, tag="H",
                                name=f"H{blk}_{s0}")
                    nc.vector.tensor_tensor(H[:], Hx, eU, OP.mult)

                    # fused Z2|Z3 reduce: E cols [0:16)=lvl2, [16:80)=lvl3
                    Z = wp.tile([GRP, ns, 20], f32, tag="Z",
                                name=f"Z{blk}_{s0}")
                    nc.vector.tensor_reduce(
                        Z[:], E.rearrange("p s (q m) -> p s q m", m=4),
                        mybir.AxisListType.X, OP.add)
                    D = wp.tile([GRP, ns, 16], f32, tag="D",
                                name=f"D{blk}_{s0}")
                    nc.vector.tensor_tensor(
                        D.rearrange("p s (j l) -> p s j l", l=4),
                        Z[:, :, 4:20].rearrange("p s (j l) -> p s j l", l=4),
                        Z[:, :, 0:4].unsqueeze(3).broadcast_to(
                            (GRP, ns, 4, 4)),
                        OP.mult)
                    R = wp.tile([GRP, ns, 16], f32, tag="R",
                                name=f"R{blk}_{s0}")
                    nc.vector.reciprocal_approx_fast(R[:], D[:])

                    m3e = wp.tile([GRP, ns, NLEAF], bf, tag="m3e",
                                  name=f"m3e{blk}_{s0}")
                    nc.vector.tensor_tensor(
                        m3e.rearrange("p s (jl m) -> p s jl m", m=4),
                        H[:].rearrange("p s (jl m) -> p s jl m", m=4),
                        R[:].unsqueeze(3).broadcast_to((GRP, ns, 16, 4)),
                        OP.mult)

                    # band tree over contiguous planes
                    if equal_w:
                        thw = th4
                    else:
                        thw = wp.tile([GRP, N_BANDS, SB, NLEAF], bf,
                                      tag="thw", name=f"thw{blk}_{s0}")
                        for n in range(N_BANDS):
                            nc.vector.tensor_scalar_mul(
                                thw[:, n, s0:s1], th4[:, n, s0:s1],
                                float(w_vec[n] * N_BANDS))
                    z01 = wp.tile([GRP, ns, NLEAF], bf, tag="z01",
                                  name=f"z01{blk}_{s0}")
                    nc.vector.tensor_tensor(
                        z01[:], thw[:, 0, s0:s1], thw[:, 1, s0:s1], OP.add)
                    z23 = wp.tile([GRP, ns, NLEAF], bf, tag="z23",
                                  name=f"z23{blk}_{s0}")
                    nc.vector.tensor_tensor(
                        z23[:], thw[:, 2, s0:s1], thw[:, 3, s0:s1], OP.add)
                    sth = wp.tile([GRP, ns, NLEAF], bf, tag="sth",
                                  name=f"sth{blk}_{s0}")
                    nc.vector.tensor_tensor(sth[:], z01[:], z23[:], OP.add)

                    pre = wp.tile([GRP, ns, NLEAF], bf, tag="pre",
                                  name=f"pre{blk}_{s0}")
                    nc.vector.scalar_tensor_tensor(
                        pre[:], sth[:], 4.0, m3e[:], OP.add, OP.mult)
                    ssum = wp.tile([GRP, ns], f32, tag="ssum",
                                   name=f"ssum{blk}_{s0}")
                    nc.vector.tensor_reduce(
                        ssum[:], pre[:], mybir.AxisListType.X, OP.add)
                    rcp = wp.tile([GRP, ns], f32, tag="rcp",
                                  name=f"rcp{blk}_{s0}")
                    nc.vector.reciprocal_approx_fast(rcp[:], ssum[:])

                    ot = wp.tile([GRP, ns, NLEAF], bf, tag="ot",
                                 name=f"ot{blk}_{s0}")
                    nc.vector.tensor_tensor(
                        ot[:], pre[:],
                        rcp[:].unsqueeze(2).broadcast_to((GRP, ns, NLEAF)),
                        OP.mult)

                    t0 = blk * BCH + s0 * GRP
                    nc.sync.dma_start(
                        out_d[t0:t0 + ns * GRP, :].rearrange(
                            "(s p) k -> p s k", p=GRP),
                        ot[:])

                if blk < NBLK - 1:
                    epilogue(0, SB)
                else:
                    # taper the last block so the final DVE chain is short
                    epilogue(0, SB // 2)
                    epilogue(SB // 2, SB)

    nc.finalize()
    return nc


def _get_compiled(equal_w, w_vec):
    key = (equal_w, tuple(np.round(w_vec.astype(np.float64), 9)))
    if key not in _compiled:
        _compiled[key] = _build_module(equal_w, w_vec)
    return _compiled[key]


def _make_in_maps(pos_3d, spectral_color, centers1, centers2, centers3,
                  portal1_T, portal2_T, W_bands, b_bands, band_weights):
    import ml_dtypes
    WA, WB, equal_w, w_vec = _host_matrices(
        np.asarray(centers1), np.asarray(centers2), np.asarray(centers3),
        np.asarray(portal1_T), np.asarray(portal2_T),
        np.asarray(W_bands), np.asarray(b_bands), np.asarray(band_weights))
    phi = _host_phi(np.asarray(pos_3d), np.asarray(spectral_color))
    bfd = ml_dtypes.bfloat16
    WAb = WA.astype(bfd)
    WBb = WB.astype(bfd)
    phib = phi.astype(bfd)
    in_maps = []
    for c in range(N_CORES):
        pc = phib[:, c * TPC:(c + 1) * TPC]           # [KB, TPC]
        # block-major: [NBLK, KB, BCH] — each block contiguous in DRAM
        pblk = np.ascontiguousarray(
            pc.reshape(KB, NBLK, BCH).transpose(1, 0, 2))
        in_maps.append({
            "phi": pblk,
            "wa": WAb,
            "wb": WBb,
        })
    return in_maps, equal_w, w_vec


def kernel(pos_3d, spectral_color, centers1, centers2, centers3,
           portal1_T, portal2_T, W_bands, b_bands, band_weights):
    from concourse.bass_utils import run_bass_kernel_spmd

    in_maps, equal_w, w_vec = _make_in_maps(
        pos_3d, spectral_color, centers1, centers2, centers3,
        portal1_T, portal2_T, W_bands, b_bands, band_weights)
    nc = _get_compiled(equal_w, w_vec)
    res = run_bass_kernel_spmd(nc, in_maps, core_ids=list(range(N_CORES)))
    outs = [np.asarray(res.results[c]["routing"], dtype=np.float32)
            for c in range(N_CORES)]
    full = np.concatenate(outs, axis=0).reshape(B, S, SPECTRAL_DIM)
    return full.astype(np.float32)


if __name__ == "__main__":
    sys.path.insert(0, "/root/problem")
    import reference
    inputs = {k: np.asarray(v) for k, v in reference.setup_inputs().items()}
    out = kernel(**inputs)
    exp = np.asarray(reference.reference(**inputs))
    err = np.max(np.abs(out - exp)) / max(np.max(np.abs(exp)), 1e-12)
    print("Relative error:", err)
